# revision 3
# baseline (speedup 1.0000x reference)
"""DisMax loss first part: logits = -(|s|*d + mean_c(|s|*d)) / temp, where
d[b,c] = ||fn_b - pn_c|| / sqrt(2) = sqrt(1 - cos(f_b, p_c)) for l2-normalized rows.

Strategy: data-parallel over the batch across 8 NeuronCores. Each core:
  [1024, 512] features x [10000, 512] prototypes -> [1024, 10000] logits.
On device (per core): normalize both operands into bf16 [d, .] layouts via
DVE square+row-sum, ACT sqrt, DVE reciprocal, DVE scale-cast, PE transposes;
then a bf16 GEMM (fp32 PSUM) in [128 x 500] chunks; ACT computes
sqrt(1 - cos) straight out of PSUM with fused row-sum accumulation; GPSIMD
applies out = dist*c0 + rowmean*c0 (c0 = -|scale|/temp); chunked 1.25 MB DMAs
stream the 40 MB result back to HBM.
"""

import sys
import types

for _p in ("/opt/trn_rl_repo", "/root/.axon_site"):
    if _p not in sys.path:
        sys.path.insert(0, _p)

# The NTFF profiling hook module is absent from this image's antenv package;
# inject the ctypes-based equivalent so trace=True works when requested.
if "antenv.axon_hooks" not in sys.modules:
    try:
        import trn_agent_boot.trn_boot as _tb

        _hook = _tb._ntff_profile_via_ctypes("/opt/axon/libaxon_pjrt.so")
        _m = types.ModuleType("antenv.axon_hooks")
        _m.get_axon_ntff_profile_hook = lambda: _hook
        sys.modules["antenv.axon_hooks"] = _m
    except Exception:
        pass

import numpy as np

import concourse.bacc as bacc
import concourse.tile as tile
import concourse.mybir as mybir
from concourse import masks
from concourse.bass_utils import run_bass_kernel_spmd

F32 = mybir.dt.float32
BF16 = mybir.dt.bfloat16
ALU = mybir.AluOpType
ACTF = mybir.ActivationFunctionType

N_CORES = 8
B, C, D = 8192, 10000, 512
BPC = B // N_CORES          # 1024 batch rows per core
NB = BPC // 128             # 8 batch tiles
ND = D // 128               # 4 contraction tiles
CCH = 500                   # matmul free-dim chunk (1 PSUM bank in f32)
NCH = C // CCH              # 20 chunks
P2 = 2500                   # pass-2 / store chunk
NP2 = C // P2               # 4
PG = 6                      # prototype tiles per staged load (1.5 MB DMA)
NPT_FULL = C // 128         # 78 full prototype tiles
P_TAIL = C - NPT_FULL * 128  # 16 rows in the tail tile
FG = 4                      # feature tiles per staged load (1 MB DMA)


def build_nc():
    nc = bacc.Bacc("TRN2", target_bir_lowering=False, debug=False,
                   num_devices=N_CORES)
    f_h = nc.dram_tensor("f", [BPC, D], F32, kind="ExternalInput")
    p_h = nc.dram_tensor("p", [C, D], F32, kind="ExternalInput")
    s_h = nc.dram_tensor("s", [1, 2], F32, kind="ExternalInput")
    o_h = nc.dram_tensor("o", [BPC, C], F32, kind="ExternalOutput")

    with tile.TileContext(nc) as tc:
        with (
            tc.tile_pool(name="const", bufs=1) as const_pool,
            tc.tile_pool(name="persist", bufs=1) as persist_pool,
            tc.tile_pool(name="fstage", bufs=2) as fstage_pool,
            tc.tile_pool(name="pstage", bufs=2) as pstage_pool,
            tc.tile_pool(name="bfc", bufs=3) as bf_pool,
            tc.tile_pool(name="sq", bufs=2) as sq_pool,
            tc.tile_pool(name="norms", bufs=2) as norm_pool,
            tc.tile_pool(name="dist", bufs=4) as dist_pool,
            tc.tile_pool(name="rs", bufs=2) as rs_pool,
            tc.tile_pool(name="ob", bufs=2) as out_pool,
            tc.tile_pool(name="ps_t", bufs=3, space="PSUM") as psum_t_pool,
            tc.tile_pool(name="ps_c", bufs=4, space="PSUM") as psum_c_pool,
            tc.tile_pool(name="ps_b", bufs=1, space="PSUM") as psum_b_pool,
        ):
            # ---- constants -------------------------------------------------
            ident = const_pool.tile([128, 128], BF16, tag="ident")
            masks.make_identity(nc, ident[:, :])
            ones = const_pool.tile([1, 128], F32, tag="ones")
            nc.vector.memset(ones[:, :], 1.0)

            # persistent bf16 transposed operands
            pnT = persist_pool.tile([128, ND, C], BF16, tag="pnT")     # 80 KB/p
            fnT = persist_pool.tile([128, ND, BPC], BF16, tag="fnT")   # 8 KB/p
            cb = persist_pool.tile([128, 2], F32, tag="cb")            # c0, c1

            # ---- scalar params: c0 = -|ds|/temp, c1 = c0/C ----------------
            stile = const_pool.tile([1, 2], F32, tag="stile")
            nc.sync.dma_start(stile[:, :], s_h[:, :])
            cv = const_pool.tile([1, 2], F32, tag="cvals")
            tmp = const_pool.tile([1, 2], F32, tag="scaltmp")
            # tmp0 = |ds|; tmp1 = 1/temp
            nc.scalar.activation(tmp[:, 0:1], stile[:, 0:1], ACTF.Abs)
            nc.vector.reciprocal(tmp[:, 1:2], stile[:, 1:2])
            # cv0 = (|ds| * -1) * (1/temp)
            nc.vector.scalar_tensor_tensor(cv[:, 0:1], tmp[:, 0:1], -1.0,
                                           tmp[:, 1:2], op0=ALU.mult,
                                           op1=ALU.mult)
            nc.vector.tensor_scalar(cv[:, 1:2], cv[:, 0:1], 1.0 / C, None,
                                    op0=ALU.mult)
            ps_b = psum_b_pool.tile([128, 2], F32, tag="psb")
            nc.tensor.matmul(ps_b[:, :], ones[:, :], cv[:, :], start=True,
                             stop=True)
            nc.vector.tensor_copy(cb[:, :], ps_b[:, :])

            # ---- feature prep ---------------------------------------------
            f_r = f_h[:, :].rearrange("(g t p) d -> g p t d", p=128, t=FG)
            for g in range(NB // FG):
                fst = fstage_pool.tile([128, FG, D], F32, tag="fst")
                nc.sync.dma_start(fst[:, :, :], f_r[g])
                fss = norm_pool.tile([128, FG], F32, tag="fss")
                finv = norm_pool.tile([128, FG], F32, tag="finv")
                for t in range(FG):
                    fsq = sq_pool.tile([128, D], BF16, tag="fsq")
                    nc.vector.scalar_tensor_tensor(
                        fsq[:, :], fst[:, t, :], 1.0, fst[:, t, :],
                        op0=ALU.mult, op1=ALU.mult,
                        accum_out=fss[:, t:t + 1])
                nc.scalar.activation(fss[:, :], fss[:, :], ACTF.Sqrt)
                nc.vector.reciprocal(finv[:, :], fss[:, :])
                for t in range(FG):
                    i = g * FG + t
                    fbf = bf_pool.tile([128, D], BF16, tag="fbf")
                    nc.vector.tensor_scalar(fbf[:, :], fst[:, t, :],
                                            finv[:, t:t + 1], None,
                                            op0=ALU.mult)
                    ps_t = psum_t_pool.tile([128, ND, 128], BF16, tag="pst")
                    for d in range(ND):
                        nc.tensor.transpose(ps_t[:, d, :],
                                            fbf[:, d * 128:(d + 1) * 128],
                                            ident[:, :])
                    nc.vector.tensor_copy(fnT[:, :, i * 128:(i + 1) * 128],
                                          ps_t[:, :, :])

            # ---- prototype prep -------------------------------------------
            n_groups = NPT_FULL // PG  # 13
            for g in range(n_groups):
                r0 = g * PG * 128
                pst = pstage_pool.tile([128, PG, D], F32, tag="pstg")
                nc.sync.dma_start(
                    pst[:, :, :],
                    p_h[r0:r0 + PG * 128, :].rearrange("(t p) d -> p t d",
                                                       p=128))
                pss = norm_pool.tile([128, PG], F32, tag="pss")
                pinv = norm_pool.tile([128, PG], F32, tag="pinv")
                for t in range(PG):
                    psq = sq_pool.tile([128, D], BF16, tag="psq")
                    nc.vector.scalar_tensor_tensor(
                        psq[:, :], pst[:, t, :], 1.0, pst[:, t, :],
                        op0=ALU.mult, op1=ALU.mult,
                        accum_out=pss[:, t:t + 1])
                nc.scalar.activation(pss[:, :], pss[:, :], ACTF.Sqrt)
                nc.vector.reciprocal(pinv[:, :], pss[:, :])
                for t in range(PG):
                    j = g * PG + t
                    pbf = bf_pool.tile([128, D], BF16, tag="pbf")
                    nc.vector.tensor_scalar(pbf[:, :], pst[:, t, :],
                                            pinv[:, t:t + 1], None,
                                            op0=ALU.mult)
                    ps_t = psum_t_pool.tile([128, ND, 128], BF16, tag="pst")
                    for d in range(ND):
                        nc.tensor.transpose(ps_t[:, d, :],
                                            pbf[:, d * 128:(d + 1) * 128],
                                            ident[:, :])
                    # alternate evacuation between DVE and ACT for balance
                    dst = pnT[:, :, j * 128:(j + 1) * 128]
                    if j % 2 == 0:
                        nc.vector.tensor_copy(dst, ps_t[:, :, :])
                    else:
                        nc.scalar.copy(dst, ps_t[:, :, :])

            # tail tile: 16 prototype rows
            r0 = NPT_FULL * 128
            ptl = pstage_pool.tile([128, D], F32, tag="ptail")
            nc.sync.dma_start(ptl[:P_TAIL, :], p_h[r0:C, :])
            pssT = norm_pool.tile([128, 1], F32, tag="pssT")
            pinvT = norm_pool.tile([128, 1], F32, tag="pinvT")
            psqT = sq_pool.tile([128, D], BF16, tag="psq")
            nc.vector.scalar_tensor_tensor(
                psqT[:P_TAIL, :], ptl[:P_TAIL, :], 1.0, ptl[:P_TAIL, :],
                op0=ALU.mult, op1=ALU.mult, accum_out=pssT[:P_TAIL, 0:1])
            nc.scalar.activation(pssT[:P_TAIL, :], pssT[:P_TAIL, :], ACTF.Sqrt)
            nc.vector.reciprocal(pinvT[:P_TAIL, :], pssT[:P_TAIL, :])
            pbfT = bf_pool.tile([128, D], BF16, tag="pbf")
            nc.vector.tensor_scalar(pbfT[:P_TAIL, :], ptl[:P_TAIL, :],
                                    pinvT[:P_TAIL, 0:1], None, op0=ALU.mult)
            ps_tT = psum_t_pool.tile([128, ND, 128], BF16, tag="pst")
            for d in range(ND):
                nc.tensor.transpose(ps_tT[:, d, :P_TAIL],
                                    pbfT[:P_TAIL, d * 128:(d + 1) * 128],
                                    ident[:P_TAIL, :P_TAIL])
            nc.vector.tensor_copy(pnT[:, :, r0:C], ps_tT[:, :, :P_TAIL])

            # ---- main loop -------------------------------------------------
            for i in range(NB):
                rs = rs_pool.tile([128, NCH], F32, tag="rs")
                dqs = []
                for q in range(NP2):
                    dq = dist_pool.tile([128, P2], F32, tag="dist")
                    dqs.append(dq)
                    for k in range(P2 // CCH):
                        ch = q * (P2 // CCH) + k
                        pc = psum_c_pool.tile([128, CCH], F32, tag="pc")
                        for d in range(ND):
                            nc.tensor.matmul(
                                pc[:, :],
                                fnT[:, d, i * 128:(i + 1) * 128],
                                pnT[:, d, ch * CCH:(ch + 1) * CCH],
                                start=(d == 0), stop=(d == ND - 1))
                        # dist = sqrt(1 - cos); accum_out = row-chunk sum
                        nc.scalar.activation(
                            dq[:, k * CCH:(k + 1) * CCH], pc[:, :],
                            ACTF.Sqrt, bias=1.0, scale=-1.0,
                            accum_out=rs[:, ch:ch + 1])
                rsum = norm_pool.tile([128, 1], F32, tag="rsum")
                bvec = norm_pool.tile([128, 1], F32, tag="bvec")
                nc.vector.reduce_sum(rsum[:, :], rs[:, :],
                                     axis=mybir.AxisListType.X)
                nc.vector.tensor_scalar(bvec[:, :], rsum[:, :], cb[:, 1:2],
                                        None, op0=ALU.mult)
                for q in range(NP2):
                    ob = out_pool.tile([128, P2], F32, tag="ob")
                    nc.gpsimd.tensor_scalar(ob[:, :], dqs[q][:, :],
                                            cb[:, 0:1], bvec[:, 0:1],
                                            op0=ALU.mult, op1=ALU.add)
                    nc.sync.dma_start(
                        o_h[i * 128:(i + 1) * 128, q * P2:(q + 1) * P2],
                        ob[:, :])

    nc.compile()
    return nc


_CACHE = {}


def _get_nc():
    if "nc" not in _CACHE:
        _CACHE["nc"] = build_nc()
    return _CACHE["nc"]


def make_in_maps(features, prototypes, distance_scale, temperature):
    f = np.ascontiguousarray(np.asarray(features, dtype=np.float32))
    p = np.ascontiguousarray(np.asarray(prototypes, dtype=np.float32))
    s = np.array([[np.float32(np.asarray(distance_scale).reshape(-1)[0]),
                   np.float32(np.asarray(temperature).reshape(-1)[0])]],
                 dtype=np.float32)
    return [
        {"f": f[i * BPC:(i + 1) * BPC], "p": p, "s": s}
        for i in range(N_CORES)
    ]


def run(features, prototypes, distance_scale, temperature, **kwargs):
    nc = _get_nc()
    in_maps = make_in_maps(features, prototypes, distance_scale, temperature)
    res = run_bass_kernel_spmd(nc, in_maps, core_ids=list(range(N_CORES)),
                               **kwargs)
    out = np.concatenate([res.results[i]["o"] for i in range(N_CORES)], axis=0)
    return out, res


def kernel(features, prototypes, distance_scale, temperature):
    out, _ = run(features, prototypes, distance_scale, temperature)
    return out
